# revision 1
# baseline (speedup 1.0000x reference)
"""Trainium2 Bass kernel for nn_CombinatorialClassifier.

Computation (reference):
    logits = einsum('bf,pqf->bpq', x, W) + b        # [B,P,Q]
    logp   = log_softmax(logits, axis=2)            # [B,P,Q]
    out    = take_along_axis(logp, part_idx, 2)     # [B,P,C]

Shapes: B=256, P=64, Q=128, C=1000, F=2048.

Sharding: expert-parallel over P across 8 cores (8 partitionings per
core).  Each core reads the full x and its W/b/part_idx slice and
writes its disjoint [B, 8, C] slice of the output.  No collectives.

Per-core dataflow ("orientation A" — q lives on SBUF partitions):
  - main matmul:   psum_lin[q, b] += WT_k[f,q].T @ xT_k[f,b], bias
    folded in as a K=1 accumulate matmul (bias[q] x ones[b]).
  - sumexp over q: ones[128,1].T @ exp[q,b] matmul (PE reduces over
    partitions), lse = Ln(sumexp) on ScalarE.
  - gather+logsoftmax in one PSUM group:
        psum_out[b, c] = linT[q,b].T @ OH[q,c] + lse[b].T @ (-1)[c]
    (the K=1 lse matmul also transposes lse into the partition dim).
    OH_p[q, c] = (q == part_idx[p,c]) is built per-p on DVE with an
    is_equal against a partition iota.

This walrus build only accepts ONE sync-wait command per compute/DMA
instruction, which dictates most of the structure:
  - x|W share one DMA per k-tile ("xw"); bias|ones share one DMA
    ("bo"), so each matmul joins on a single semaphore.
  - every SBUF tile is used exactly once (fresh slot) -> no
    WAR/WAW slot-release waits anywhere.
  - idx and iota for partitioning p are DMAd back-to-back so the
    SWDGE round-robin lands them on the same queue semaphore; the
    is_equal TT then joins on that one sem.
  - PSUM->SBUF result copies all run on DVE into per-(p-pair,bt)
    group tiles; each output DMA (on the ACT HWDGE) is preceded by a
    tiny ACT "observer" op that absorbs the DVE producer wait, so the
    DMA itself only carries its queue-predecessor wait.
  - bf16 for x/W (also halves their HBM traffic); the gather path is
    float32r (full-rate PE fp32).
"""

import numpy as np

B, P, Q, C, F = 256, 64, 128, 1000, 2048
NCORES = 8
PL = P // NCORES          # partitionings per core
KT = F // 128             # contraction tiles
BT = B // 128             # batch tiles for the gather matmul
C_CHUNKS = [(0, 512), (512, C - 512)]

MAIN_BF16 = True          # store/stream x,W as bf16 and matmul in bf16
GATHER_R = True           # gather/lse/sumexp matmul operands in float32r


def _build_nc():
    import concourse.bass as bass
    import concourse.tile as tile
    from concourse import mybir
    from contextlib import ExitStack

    DT = mybir.dt.float32
    HT = mybir.dt.float16
    MDT = mybir.dt.bfloat16 if MAIN_BF16 else mybir.dt.float32r
    # bf16 gather operands: 2-byte stationary loads keep the PE at full
    # rate (fp32r 4-byte weight loads measured ~2x slower per matmul)
    GDT = mybir.dt.bfloat16

    nc = bass.Bass()
    xw_d = nc.declare_dram_parameter("xw", [KT, 128, B + PL * Q], MDT,
                                     isOutput=False)
    bo_d = nc.declare_dram_parameter("bo", [1, PL * Q + B], MDT,
                                     isOutput=False)
    # idxq[q, p, :C] = part_idx[p, :] (same on every partition row) and
    # idxq[q, p, C] = q — idx and iota in ONE tensor/DMA, so the
    # is_equal TT joins on a single DMA semaphore
    idx_d = nc.declare_dram_parameter("idxq", [Q, PL, C + 1], HT,
                                      isOutput=False)
    out_d = nc.declare_dram_parameter("out", [B, PL, C], DT, isOutput=True)

    with ExitStack() as ctx:
        tc = ctx.enter_context(tile.TileContext(nc))
        singles = ctx.enter_context(tc.tile_pool(name="singles", bufs=1))
        ps_lin = ctx.enter_context(
            tc.tile_pool(name="ps_lin", bufs=2, space=bass.MemorySpace.PSUM))
        ps_sum = ctx.enter_context(
            tc.tile_pool(name="ps_sum", bufs=2, space=bass.MemorySpace.PSUM))
        ps_out = ctx.enter_context(
            tc.tile_pool(name="ps_out", bufs=4, space=bass.MemorySpace.PSUM))

        def fresh(shape, dtype, tag):
            return singles.tile(shape, dtype, tag=tag, name=tag)

        # ---- static tiles (all fresh, single-use) -------------------
        xwk = []
        for k in range(KT):
            t = fresh([128, B + PL * Q], MDT, f"xwk{k}")
            nc.sync.dma_start(out=t[:], in_=xw_d[k])
            xwk.append(t)
        bo_sb = fresh([1, PL * Q + B], MDT, "bo")
        nc.sync.dma_start(out=bo_sb[:], in_=bo_d[:])
        idx_sb = fresh([128, PL, C + 1], HT, "idxq")
        nc.sync.dma_start(out=idx_sb[:], in_=idx_d[:])

        # ACT-produced constants so the ACT-side matmuls join on ACT
        ones_col = fresh([128, 1], GDT, "ones")
        nc.scalar.activation(out=ones_col[:], in_=xwk[0][:, 0:1],
                             func=mybir.ActivationFunctionType.Copy,
                             bias=1.0, scale=0.0)
        negones_sb = fresh([1, 512], GDT, "negones")
        nc.scalar.activation(out=negones_sb[:], in_=bo_sb[0:1, 0:512],
                             func=mybir.ActivationFunctionType.Copy,
                             bias=-1.0, scale=0.0)

        obs_scratch = fresh([1, 4 * PL], DT, "obs")

        # ---- per-partitioning pipeline ------------------------------
        og_tiles = {}
        n_obs = 0
        for p in range(PL):
            psum_lin = ps_lin.tile([128, B], DT)
            # bias: K=1 matmul bias[q] x ones[b] opens the accumulation
            nc.tensor.matmul(
                psum_lin[:],
                bo_sb[:, p * Q:(p + 1) * Q],
                bo_sb[:, PL * Q:],
                start=True, stop=False)
            for k in range(KT):
                nc.tensor.matmul(
                    psum_lin[:],
                    xwk[k][:, B + p * Q:B + (p + 1) * Q],
                    xwk[k][:, :B],
                    start=False,
                    stop=(k == KT - 1),
                )

            # one-hot build for this p on DVE (single DMA sem join)
            oh_p = fresh([128, C], GDT, f"oh{p}")
            nc.vector.tensor_tensor(
                out=oh_p[:],
                in0=idx_sb[:, p, :C],
                in1=idx_sb[:, p, C:C + 1].broadcast_to((128, C)),
                op=mybir.AluOpType.is_equal,
            )

            linT = fresh([128, B], GDT, f"lin{p}")
            nc.vector.tensor_copy(linT[:], psum_lin[:])
            expT = fresh([128, B], GDT, f"exp{p}")
            nc.scalar.activation(
                out=expT[:], in_=linT[:],
                func=mybir.ActivationFunctionType.Exp)

            psum_sum = ps_sum.tile([1, B], DT)
            nc.tensor.matmul(
                psum_sum[:], ones_col[:], expT[:],
                start=True, stop=True)
            lse = fresh([1, B], GDT, f"lse{p}")
            nc.scalar.activation(
                out=lse[:], in_=psum_sum[:],
                func=mybir.ActivationFunctionType.Ln)

            pair = p // 2
            for bt in range(BT):
                bsl = slice(bt * 128, (bt + 1) * 128)
                if p % 2 == 0:
                    og_new = fresh([128, 2, C], DT, f"og{pair}_{bt}")
                    og_tiles[(pair, bt)] = og_new
                og = og_tiles[(pair, bt)]
                last_copy = None
                for (c0, cw) in C_CHUNKS:
                    psum_out = ps_out.tile([128, 512], DT)
                    nc.tensor.matmul(
                        psum_out[:, :cw],
                        linT[:, bsl],
                        oh_p[:, c0:c0 + cw],
                        start=True, stop=False)
                    nc.tensor.matmul(
                        psum_out[:, :cw],
                        lse[:, bsl],
                        negones_sb[:, :cw],
                        start=False, stop=True)
                    last_copy = nc.vector.tensor_copy(
                        og[:, p % 2, c0:c0 + cw], psum_out[:, :cw])
                if p % 2 == 1:
                    # ACT observer absorbs the DVE producer wait; the
                    # DMA then only carries its queue-predecessor wait
                    obs = nc.scalar.activation(
                        out=obs_scratch[0:1, n_obs:n_obs + 1],
                        in_=og[0:1, 1, C - 1:C],
                        func=mybir.ActivationFunctionType.Copy,
                        bias=0.0, scale=1.0)
                    n_obs += 1
                    dma = nc.scalar.dma_start(
                        out=out_d[bsl, p - 1:p + 1, :],
                        in_=og[:])
                    tile.add_dep_helper(dma.ins, obs.ins, sync=False,
                                        reason="dma after observer")

    _install_drain_split(nc)
    return nc


def _install_drain_split(nc, chunk=1):
    """The kernel-tail Drain waits on every live semaphore (~11), but
    this walrus build's CTRL_NO encoding fits only a couple of sync
    commands.  Splitting the drain into a chain of drains, each
    carrying `chunk` waits, is semantically identical (sequential SP
    sem waits).  Patch at serialization time so every consumer of
    nc.to_json_bytes() sees the legal form."""
    import copy
    import json

    orig = nc.to_json_bytes

    def patched():
        m = json.loads(orig())
        for fn in m["functions"]:
            for bb in fn["blocks"]:
                out = []
                for inst in bb["instructions"]:
                    si = inst.get("sync_info")
                    if (inst.get("opcode") == "Drain" and si
                            and si.get("on_wait")
                            and len(si["on_wait"]) > chunk):
                        waits = si["on_wait"]
                        head, keep = waits[:-chunk], waits[-chunk:]
                        for j in range(0, len(head), chunk):
                            clone = copy.deepcopy(inst)
                            clone["name"] = f"{inst['name']}-ds{j}"
                            clone["sync_info"] = {
                                "on_wait": head[j:j + chunk],
                                "on_update": [],
                            }
                            out.append(clone)
                        si["on_wait"] = keep
                    out.append(inst)
                bb["instructions"] = out
        return json.dumps(m).encode()

    nc.to_json_bytes = patched


def _host_inputs(x, W, b, part_idx):
    """Build the 8 per-core input maps."""
    import ml_dtypes

    mm_np = ml_dtypes.bfloat16 if MAIN_BF16 else np.float32
    xT = x.T.reshape(KT, 128, B).astype(mm_np)                # [KT,128,B]
    in_maps = []
    for i in range(NCORES):
        sl = slice(i * PL, (i + 1) * PL)
        WT = W[sl].transpose(2, 0, 1).reshape(
            KT, 128, PL * Q).astype(mm_np)                    # [KT,128,PL*Q]
        xw = np.empty((KT, 128, B + PL * Q), dtype=mm_np)
        xw[:, :, :B] = xT
        xw[:, :, B:] = WT
        bo = np.empty((1, PL * Q + B), dtype=mm_np)
        bo[0, :PL * Q] = b[sl].reshape(-1)
        bo[0, PL * Q:] = 1.0
        idxq = np.empty((Q, PL, C + 1), dtype=np.float16)
        idxq[:, :, :C] = part_idx[sl].astype(np.float16)[None, :, :]
        idxq[:, :, C] = np.arange(Q, dtype=np.float16)[:, None]
        in_maps.append({"xw": xw, "bo": bo, "idxq": idxq})
    return in_maps


def kernel(x, W, b, part_idx, _trace=False):
    from concourse.bass_utils import run_bass_kernel_spmd

    x = np.asarray(x, dtype=np.float32)
    W = np.asarray(W, dtype=np.float32)
    b = np.asarray(b, dtype=np.float32)
    part_idx = np.asarray(part_idx)

    nc = _build_nc()
    in_maps = _host_inputs(x, W, b, part_idx)
    res = run_bass_kernel_spmd(nc, in_maps, list(range(NCORES)),
                               trace=_trace)
    out = np.concatenate([r["out"] for r in res.results], axis=1)
    if _trace:
        return out, res
    return out



# revision 6
# speedup vs baseline: 1.1083x; 1.1083x over previous
"""Trainium2 Bass kernel for nn_CombinatorialClassifier (v2).

Computation (reference):
    logits = einsum('bf,pqf->bpq', x, W) + b        # [B,P,Q]
    logp   = log_softmax(logits, axis=2)            # [B,P,Q]
    out    = take_along_axis(logp, part_idx, 2)     # [B,P,C]

Shapes: B=256, P=64, Q=128, C=1000, F=2048.  Expert-parallel over P
across 8 cores (PL=8 partitionings per core), no collectives.

v2 design (vs baseline):
  - main matmul in "b-orientation": stationary = xT k-slab [128f, 128b],
    moving = W k-slab [128f, 1024(p,q)] -> psum_lin[b, (p,q)].  W
    streams k-tile by k-tile from HBM and each tile feeds its matmuls
    immediately (the baseline stalled PE ~20us waiting for the whole W).
    64 N=512 matmuls instead of 128 N=256 ones.
  - W in fp8e4 (x64 scale folded out later) halves W HBM traffic.
  - log-softmax folded BEFORE the gather: logp = psum/64 - lse via one
    fused scalar_tensor_tensor per batch-half; the gather matmul then
    needs no K=1 lse-broadcast matmuls (baseline spent ~16k PE cycles
    on those) and PSUM drains are plain copies.
  - sumexp via ACT Exp + DVE free-axis segmented reduce (q lives on the
    free dim in this orientation).
  - logp transposed back to [q, b] with 8 PE transposes per bt into one
    bf16 PSUM bank, single DVE copy out.
  - output written as bf16 (tolerance 2e-2 >> bf16 eps) halving out DMA.
  - drains split DVE (p even) / ACT (p odd); out-DMA per p-pair with an
    ACT observer absorbing the DVE-side producer wait.
  - one-hot build on GpSimd (otherwise idle), covered for PE consumers
    by a tiny DVE relay before linT_bt0.
"""

import numpy as np

B, P, Q, C, F = 256, 64, 128, 1000, 2048
NCORES = 8
PL = P // NCORES          # partitionings per core
KT = F // 128             # contraction k-tiles
SCALE = 64.0              # W pre-scale to keep fp8e4 out of subnormals
W_FP8 = True
OH_GPSIMD = False     # Pool engine rejects TensorTensor in this ISA


def _build_nc():
    import concourse.bass as bass
    import concourse.tile as tile
    from concourse import mybir
    from contextlib import ExitStack

    F32 = mybir.dt.float32
    BF16 = mybir.dt.bfloat16
    FP16 = mybir.dt.float16
    WDT = mybir.dt.float8e4 if W_FP8 else mybir.dt.bfloat16
    AF = mybir.ActivationFunctionType
    ALU = mybir.AluOpType

    nc = bass.Bass()
    xT_d = nc.declare_dram_parameter("xT", [128, KT * 256], BF16,
                                     isOutput=False)
    bias_d = nc.declare_dram_parameter("biasr", [1, PL * Q + 128], BF16,
                                       isOutput=False)
    id_d = nc.declare_dram_parameter("ident", [128, 128], BF16,
                                     isOutput=False)
    wm_d = nc.declare_dram_parameter("wm", [KT, 128, PL * Q], WDT,
                                     isOutput=False)
    idx_d = nc.declare_dram_parameter("idxq", [128, PL, C + 1], FP16,
                                      isOutput=False)
    out_d = nc.declare_dram_parameter("out", [B, PL, C], BF16, isOutput=True)

    with ExitStack() as ctx:
        tc = ctx.enter_context(tile.TileContext(nc))
        sb = ctx.enter_context(tc.tile_pool(name="sb", bufs=1))
        # [128, 1024] f32 = 2 PSUM banks per slot; lin_bt0/lin_bt1 and all
        # gather outputs rotate through 3 slots (6 banks)
        ps_big = ctx.enter_context(
            tc.tile_pool(name="ps_big", bufs=3, space=bass.MemorySpace.PSUM))
        # PSUM stores 32-bit words, so even the bf16 transpose tile costs
        # 2 banks; single slot shared by both bt (WAR is DVE-to-DVE)
        ps_tr = ctx.enter_context(
            tc.tile_pool(name="ps_tr", bufs=1, space=bass.MemorySpace.PSUM))

        def fresh(shape, dtype, tag):
            return sb.tile(shape, dtype, tag=tag, name=tag)

        # ---- input DMAs (sync HWDGE family), consumption order -------
        xT = fresh([128, KT * 256], BF16, "xT")
        nc.sync.dma_start(out=xT[:], in_=xT_d[:])
        biasr = fresh([1, PL * Q + 128], BF16, "biasr")
        nc.sync.dma_start(out=biasr[:], in_=bias_d[:])
        ident = fresh([128, 128], BF16, "ident")
        nc.sync.dma_start(out=ident[:], in_=id_d[:])
        wk = []
        for k in range(KT):
            t = fresh([128, PL * Q], WDT, f"wk{k}")
            nc.sync.dma_start(out=t[:], in_=wm_d[k])
            wk.append(t)
        idxq = fresh([128, PL, C + 1], FP16, "idxq")
        nc.sync.dma_start(out=idxq[:], in_=idx_d[:])

        # ---- one-hot per p (runs during the main phase) --------------
        oh = []
        oh_eng = nc.gpsimd if OH_GPSIMD else nc.vector
        for p in range(PL):
            t = fresh([128, C], BF16, f"oh{p}")
            oh_eng.tensor_tensor(
                out=t[:], in0=idxq[:, p, :C],
                in1=idxq[:, p, C:C + 1].broadcast_to((128, C)),
                op=ALU.is_equal)
            oh.append(t)

        # ---- main matmuls: psum_lin[b, (p,q)] ------------------------
        lin = [ps_big.tile([128, PL, 128], F32, tag="big", name=f"lin{bt}")
               for bt in (0, 1)]
        ones_ap = biasr[:, PL * Q:PL * Q + 128]
        for bt in (0, 1):
            for ch in (0, 1):
                nc.tensor.matmul(
                    lin[bt][:, ch * 4:(ch + 1) * 4, :],
                    ones_ap, biasr[:, ch * 512:(ch + 1) * 512],
                    start=True, stop=False)
        for k in range(KT):
            for bt in (0, 1):
                for ch in (0, 1):
                    nc.tensor.matmul(
                        lin[bt][:, ch * 4:(ch + 1) * 4, :],
                        xT[:, k * 256 + bt * 128:k * 256 + (bt + 1) * 128],
                        wk[k][:, ch * 512:(ch + 1) * 512],
                        start=False, stop=(k == KT - 1))

        # ---- per-bt softmax chain ------------------------------------
        logpY = []
        for bt in (0, 1):
            exps = fresh([128, PL, 128], BF16, f"exps{bt}")
            nc.scalar.activation(out=exps[:], in_=lin[bt][:], func=AF.Exp,
                                 scale=1.0 / SCALE)
            sums = fresh([128, PL], F32, f"sums{bt}")
            nc.vector.tensor_reduce(out=sums[:], in_=exps[:],
                                    axis=mybir.AxisListType.X, op=ALU.add)
            lse = fresh([128, PL], F32, f"lse{bt}")
            nc.scalar.activation(out=lse[:], in_=sums[:], func=AF.Ln)
            lp = fresh([128, PL, 128], BF16, f"logpY{bt}")
            nc.vector.scalar_tensor_tensor(
                out=lp[:], in0=lin[bt][:], scalar=1.0 / SCALE,
                in1=lse[:].unsqueeze(2).broadcast_to((128, PL, 128)),
                op0=ALU.mult, op1=ALU.subtract)
            logpY.append(lp)

        # DVE relay: cover gpsimd-built one-hots for the PE gather
        # consumers (PE then only ever waits on DVE for them)
        if OH_GPSIMD:
            relay = fresh([1, 1], BF16, "relay")
            nc.vector.tensor_copy(out=relay[:], in_=oh[PL - 1][0:1, 0:1])

        # ---- per-bt: transpose -> gather -> drain -> out DMA ---------
        obs_scratch = fresh([1, 16], F32, "obs")
        n_obs = 0
        og = {}
        for bt in (0, 1):
            tr = ps_tr.tile([128, PL, 128], BF16, name=f"tr{bt}")
            for p in range(PL):
                nc.tensor.transpose(tr[:, p, :], logpY[bt][:, p, :],
                                    ident[:])
            linT = fresh([128, PL, 128], BF16, f"linT{bt}")
            nc.vector.tensor_copy(out=linT[:], in_=tr[:])

            for p in range(PL):
                pair = p // 2
                po = ps_big.tile([128, 1024], F32, tag="big",
                                 name=f"po{bt}_{p}")
                nc.tensor.matmul(po[:, 0:512], linT[:, p, :],
                                 oh[p][:, 0:512], start=True, stop=True)
                nc.tensor.matmul(po[:, 512:1000], linT[:, p, :],
                                 oh[p][:, 512:1000], start=True, stop=True)
                if p % 2 == 0:
                    og[(pair, bt)] = fresh([128, 2, C], BF16, f"og{pair}_{bt}")
                g = og[(pair, bt)]
                if p % 2 == 0:
                    nc.vector.tensor_copy(out=g[:, 0, :], in_=po[:, 0:1000])
                else:
                    nc.scalar.activation(out=g[:, 1, :], in_=po[:, 0:1000],
                                         func=AF.Copy)
                    # ACT observer reads the DVE-drained half so the out
                    # DMA (scalar queue) carries no cross-engine wait
                    obs = nc.scalar.activation(
                        out=obs_scratch[0:1, n_obs:n_obs + 1],
                        in_=g[0:1, 0, C - 1:C], func=AF.Copy)
                    n_obs += 1
                    bsl = slice(bt * 128, (bt + 1) * 128)
                    dma = nc.scalar.dma_start(
                        out=out_d[bsl, p - 1:p + 1, :], in_=g[:])
                    tile.add_dep_helper(dma.ins, obs.ins, sync=False,
                                        reason="dma after observer")

    _install_wait_split(nc)
    return nc


def _install_wait_split(nc):
    """This walrus build encodes at most ONE sync-wait per instruction.
    Legalize at serialization time: any instruction carrying N>1 waits
    gets N-1 wait-only Drain instructions (same engine, so the queue
    stalls identically) inserted in front of it; the instruction keeps
    the last wait.  Semantically identical (serial sem waits)."""
    import json

    orig = nc.to_json_bytes

    def patched():
        m = json.loads(orig())
        for fn in m["functions"]:
            for bb in fn["blocks"]:
                out = []
                for inst in bb["instructions"]:
                    si = inst.get("sync_info")
                    waits = (si or {}).get("on_wait") or []
                    if len(waits) > 1:
                        head, keep = waits[:-1], waits[-1:]
                        for j, w in enumerate(head):
                            out.append({
                                "engine": inst["engine"],
                                "ins": [],
                                "outs": [],
                                "name": f"{inst['name']}-ws{j}",
                                "opcode": "Drain",
                                "sync_info": {
                                    "on_wait": [w],
                                    "on_update": [],
                                },
                            })
                        si["on_wait"] = keep
                    out.append(inst)
                bb["instructions"] = out
        return json.dumps(m).encode()

    nc.to_json_bytes = patched


def _host_inputs(x, W, b, part_idx):
    import ml_dtypes

    wnp = ml_dtypes.float8_e4m3 if W_FP8 else ml_dtypes.bfloat16
    bf = ml_dtypes.bfloat16

    # xT[f_sub, k*256 + b] = x[b, k*128 + f_sub]
    xT = np.ascontiguousarray(
        x.reshape(B, KT, 128).transpose(2, 1, 0).reshape(128, KT * B)
    ).astype(bf)
    ident = np.eye(128, dtype=np.float32).astype(bf)

    in_maps = []
    for i in range(NCORES):
        sl = slice(i * PL, (i + 1) * PL)
        # wm[k, f_sub, p*128+q] = SCALE * W[p, q, k*128+f_sub]
        wm = np.ascontiguousarray(
            (W[sl] * SCALE).transpose(2, 0, 1)
            .reshape(KT, 128, PL * Q)).astype(wnp)
        biasr = np.empty((1, PL * Q + 128), dtype=bf)
        biasr[0, :PL * Q] = (b[sl] * SCALE).reshape(-1).astype(bf)
        biasr[0, PL * Q:] = 1.0
        idxq = np.empty((128, PL, C + 1), dtype=np.float16)
        idxq[:, :, :C] = part_idx[sl].astype(np.float16)[None, :, :]
        idxq[:, :, C] = np.arange(128, dtype=np.float16)[:, None]
        in_maps.append({"xT": xT, "biasr": biasr, "ident": ident,
                        "wm": wm, "idxq": idxq})
    return in_maps


def kernel(x, W, b, part_idx, _trace=False):
    from concourse.bass_utils import run_bass_kernel_spmd

    x = np.asarray(x, dtype=np.float32)
    W = np.asarray(W, dtype=np.float32)
    b = np.asarray(b, dtype=np.float32)
    part_idx = np.asarray(part_idx)

    nc = _build_nc()
    in_maps = _host_inputs(x, W, b, part_idx)
    res = run_bass_kernel_spmd(nc, in_maps, list(range(NCORES)),
                               trace=_trace)
    out = np.concatenate(
        [np.asarray(r["out"], dtype=np.float32) for r in res.results], axis=1)
    if _trace:
        return out, res
    return out


# revision 7
# speedup vs baseline: 1.3031x; 1.1758x over previous
"""Trainium2 Bass kernel for nn_CombinatorialClassifier (v3).

Computation (reference):
    logits = einsum('bf,pqf->bpq', x, W) + b        # [B,P,Q]
    logp   = log_softmax(logits, axis=2)            # [B,P,Q]
    out    = take_along_axis(logp, part_idx, 2)     # [B,P,C]

Shapes: B=256, P=64, Q=128, C=1000, F=2048.  Expert-parallel over P
across 8 cores (PL=8 partitionings per core), no collectives.

Key structure:
  - main matmul in "b-orientation": stationary = xT k-slab [128f, 128b],
    moving = W k-slab [128f, (p,q)-chunk] -> psum_lin[b, (p,q)].
    W streams from HBM in 4 chunks of 4 k-tiles and feeds matmuls
    immediately; x/W in fp8e4 (W pre-scaled x64, folded back out in the
    softmax) so the whole input stream is ~3.1MB.
  - PE warm-up: the HAM clock gate defaults the PE to 1.2 GHz and only
    sustained busy flips it to 2.4 GHz.  A dozen junk matmuls at t=0
    (while DMAs stream) warm it; a few more "warmkeeper" matmuls after
    the main phase bridge the softmax-chain gap so the tail runs warm.
  - log-softmax folded BEFORE the gather: one fused scalar_tensor_tensor
    per batch-half computes logp = psum/64 - lse; gather PSUM drains are
    then plain copies, split DVE/ACT.
  - sumexp via ACT Exp + DVE segmented free-axis reduce.
  - logp transposed back to [q, b] with 8 PE transposes per bt into one
    bf16 PSUM bank, one DVE copy out; gather = logpT.T @ one-hot.
  - one-hot: idx replicated across partitions by DMA (fp16, fused with a
    host-built iota block, split in two DMAs so it lands early) then
    8 DVE is_equal ops in 2x mode against the dense iota tile.
  - output bf16 (tolerance 2e-2 >> bf16 eps): 4.1MB out DMA, issued
    alternately from the sync and scalar HWDGE families.
  - _install_wait_split legalizes multi-wait instructions for this
    walrus build (max one sync-wait per instruction) by prepending
    same-engine wait-only Drains.
"""

import numpy as np

B, P, Q, C, F = 256, 64, 128, 1000, 2048
NCORES = 8
PL = P // NCORES          # partitionings per core
KT = F // 128             # contraction k-tiles
KC = 4                    # k-tiles per W DMA chunk
SCALE = 64.0              # W pre-scale keeps fp8e4 out of subnormals
N_WARM = 12               # junk matmuls at t=0 (PE HAM warm-up)
N_KEEP = 6                # junk matmuls after main (keep PE warm)


def _build_nc():
    import concourse.bass as bass
    import concourse.tile as tile
    from concourse import mybir
    from contextlib import ExitStack

    F32 = mybir.dt.float32
    BF16 = mybir.dt.bfloat16
    FP16 = mybir.dt.float16
    FP8 = mybir.dt.float8e4
    AF = mybir.ActivationFunctionType
    ALU = mybir.AluOpType

    nc = bass.Bass()
    xT_d = nc.declare_dram_parameter("xT", [128, KT * 256], FP8,
                                     isOutput=False)
    bias_d = nc.declare_dram_parameter("biasr", [1, PL * Q + 128], BF16,
                                       isOutput=False)
    id_d = nc.declare_dram_parameter("ident", [128, 128], BF16,
                                     isOutput=False)
    wm_d = nc.declare_dram_parameter("wm", [KT // KC, 128, KC * PL * Q], FP8,
                                     isOutput=False)
    # cols [0,1000) = iota (row q has value q), [1000+p*1000, ...) = idx[p]
    idx_d = nc.declare_dram_parameter("idxq", [128, (PL + 1) * C], FP16,
                                      isOutput=False)
    out_d = nc.declare_dram_parameter("out", [B, PL, C], BF16, isOutput=True)

    with ExitStack() as ctx:
        tc = ctx.enter_context(tile.TileContext(nc))
        sb = ctx.enter_context(tc.tile_pool(name="sb", bufs=1))
        # [128, 1024] f32 = 2 PSUM banks/slot; warmup + lin_bt + gather
        # outputs rotate through 3 slots (6 banks)
        ps_big = ctx.enter_context(
            tc.tile_pool(name="ps_big", bufs=3, space=bass.MemorySpace.PSUM))
        # transpose target (PSUM is 32-bit-word backed: 2 banks even bf16)
        ps_tr = ctx.enter_context(
            tc.tile_pool(name="ps_tr", bufs=1, space=bass.MemorySpace.PSUM))

        def fresh(shape, dtype, tag):
            return sb.tile(shape, dtype, tag=tag, name=tag)

        # ---- PE warm-up (no input deps beyond a gpsimd memset) -------
        wu = fresh([128, 512], BF16, "wu")
        nc.gpsimd.memset(wu[:], 0)
        wu_ps = ps_big.tile([128, 1024], F32, tag="big", name="wu_ps")
        for _ in range(N_WARM):
            nc.tensor.matmul(wu_ps[:, 0:512], wu[:, 0:128], wu[:, :],
                             start=True, stop=True)

        # ---- input DMAs (sync HWDGE family), consumption order -------
        xT = fresh([128, KT * 256], FP8, "xT")
        nc.sync.dma_start(out=xT[:], in_=xT_d[:])
        biasr = fresh([1, PL * Q + 128], BF16, "biasr")
        nc.sync.dma_start(out=biasr[:], in_=bias_d[:])
        ident = fresh([128, 128], BF16, "ident")
        nc.sync.dma_start(out=ident[:], in_=id_d[:])
        wkc = []
        idx_sb = fresh([128, (PL + 1) * C], FP16, "idxq")
        for j in range(KT // KC):
            t = fresh([128, KC * PL * Q], FP8, f"wk{j}")
            nc.sync.dma_start(out=t[:], in_=wm_d[j])
            wkc.append(t)
            # interleave the two idx DMAs early in the stream
            if j == 0:
                nc.sync.dma_start(out=idx_sb[:, 0:5 * C],
                                  in_=idx_d[:, 0:5 * C])
            elif j == 1:
                nc.sync.dma_start(out=idx_sb[:, 5 * C:],
                                  in_=idx_d[:, 5 * C:])

        # ---- one-hot per p: 2x-mode DVE is_equal against dense iota --
        oh = []
        for p in range(PL):
            t = fresh([128, C], BF16, f"oh{p}")
            nc.vector.tensor_tensor(
                out=t[:], in0=idx_sb[:, (1 + p) * C:(2 + p) * C],
                in1=idx_sb[:, 0:C], op=ALU.is_equal)
            oh.append(t)

        # ---- main matmuls: psum_lin[b, (p,q)] ------------------------
        lin = [ps_big.tile([128, PL, 128], F32, tag="big", name=f"lin{bt}")
               for bt in (0, 1)]
        ones_ap = biasr[:, PL * Q:PL * Q + 128]
        for bt in (0, 1):
            for ch in (0, 1):
                nc.tensor.matmul(
                    lin[bt][:, ch * 4:(ch + 1) * 4, :],
                    ones_ap, biasr[:, ch * 512:(ch + 1) * 512],
                    start=True, stop=False)
        for k in range(KT):
            j, kk = k // KC, k % KC
            for bt in (0, 1):
                for ch in (0, 1):
                    nc.tensor.matmul(
                        lin[bt][:, ch * 4:(ch + 1) * 4, :],
                        xT[:, k * 256 + bt * 128:k * 256 + (bt + 1) * 128],
                        wkc[j][:, kk * 1024 + ch * 512:
                               kk * 1024 + (ch + 1) * 512],
                        start=False, stop=(k == KT - 1))

        # ---- keep the PE warm across the softmax-chain gap -----------
        for _ in range(N_KEEP):
            nc.tensor.matmul(wu_ps[:, 0:512], wu[:, 0:128], wu[:, :],
                             start=True, stop=True)

        # ---- per-bt softmax chain ------------------------------------
        logpY = []
        for bt in (0, 1):
            exps = fresh([128, PL, 128], BF16, f"exps{bt}")
            nc.scalar.activation(out=exps[:], in_=lin[bt][:], func=AF.Exp,
                                 scale=1.0 / SCALE)
            sums = fresh([128, PL], F32, f"sums{bt}")
            nc.vector.tensor_reduce(out=sums[:], in_=exps[:],
                                    axis=mybir.AxisListType.X, op=ALU.add)
            lse = fresh([128, PL], F32, f"lse{bt}")
            nc.scalar.activation(out=lse[:], in_=sums[:], func=AF.Ln)
            lp = fresh([128, PL, 128], BF16, f"logpY{bt}")
            nc.vector.scalar_tensor_tensor(
                out=lp[:], in0=lin[bt][:], scalar=1.0 / SCALE,
                in1=lse[:].unsqueeze(2).broadcast_to((128, PL, 128)),
                op0=ALU.mult, op1=ALU.subtract)
            logpY.append(lp)

        # ---- per-bt: transpose -> gather -> drain -> out DMA ---------
        og = {}
        for bt in (0, 1):
            tr = ps_tr.tile([128, PL, 128], BF16, name=f"tr{bt}")
            for p in range(PL):
                nc.tensor.transpose(tr[:, p, :], logpY[bt][:, p, :],
                                    ident[:])
            linT = fresh([128, PL, 128], BF16, f"linT{bt}")
            nc.vector.tensor_copy(out=linT[:], in_=tr[:])

            for p in range(PL):
                pair = p // 2
                po = ps_big.tile([128, 1024], F32, tag="big",
                                 name=f"po{bt}_{p}")
                nc.tensor.matmul(po[:, 0:512], linT[:, p, :],
                                 oh[p][:, 0:512], start=True, stop=True)
                nc.tensor.matmul(po[:, 512:1000], linT[:, p, :],
                                 oh[p][:, 512:1000], start=True, stop=True)
                if p % 2 == 0:
                    og[(pair, bt)] = fresh([128, 2, C], BF16, f"og{pair}_{bt}")
                g = og[(pair, bt)]
                if p % 2 == 0:
                    nc.vector.tensor_copy(out=g[:, 0, :], in_=po[:, 0:1000])
                else:
                    nc.scalar.activation(out=g[:, 1, :], in_=po[:, 0:1000],
                                         func=AF.Copy)
                    bsl = slice(bt * 128, (bt + 1) * 128)
                    eng = nc.scalar if pair % 2 == 0 else nc.sync
                    eng.dma_start(out=out_d[bsl, p - 1:p + 1, :], in_=g[:])

    _install_wait_split(nc)
    return nc


def _install_wait_split(nc):
    """This walrus build encodes at most ONE sync-wait per instruction.
    Legalize at serialization time: any instruction carrying N>1 waits
    gets N-1 wait-only Drain instructions (same engine, so the queue
    stalls identically) inserted in front of it; the instruction keeps
    the last wait.  Semantically identical (serial sem waits)."""
    import json

    orig = nc.to_json_bytes

    def patched():
        m = json.loads(orig())
        for fn in m["functions"]:
            for bb in fn["blocks"]:
                out = []
                for inst in bb["instructions"]:
                    si = inst.get("sync_info")
                    waits = (si or {}).get("on_wait") or []
                    if len(waits) > 1:
                        head, keep = waits[:-1], waits[-1:]
                        for j, w in enumerate(head):
                            out.append({
                                "engine": inst["engine"],
                                "ins": [],
                                "outs": [],
                                "name": f"{inst['name']}-ws{j}",
                                "opcode": "Drain",
                                "sync_info": {
                                    "on_wait": [w],
                                    "on_update": [],
                                },
                            })
                        si["on_wait"] = keep
                    out.append(inst)
                bb["instructions"] = out
        return json.dumps(m).encode()

    nc.to_json_bytes = patched


def _host_inputs(x, W, b, part_idx):
    import ml_dtypes

    f8 = ml_dtypes.float8_e4m3
    bf = ml_dtypes.bfloat16

    # xT[f_sub, k*256 + b] = x[b, k*128 + f_sub]
    xT = np.ascontiguousarray(
        x.reshape(B, KT, 128).transpose(2, 1, 0).reshape(128, KT * B)
    ).astype(f8)
    ident = np.eye(128, dtype=np.float32).astype(bf)
    iota = np.arange(128, dtype=np.float16)

    in_maps = []
    for i in range(NCORES):
        sl = slice(i * PL, (i + 1) * PL)
        # wm[j, f_sub, kk*1024 + p*128 + q] = SCALE * W[p, q, (j*KC+kk)*128+f]
        wm = np.ascontiguousarray(
            (W[sl] * SCALE).transpose(2, 0, 1)          # [F, PL, Q]
            .reshape(KT // KC, KC * 128, PL * Q)
            .reshape(KT // KC, KC, 128, PL * Q)
            .transpose(0, 2, 1, 3)                      # [J, 128, KC, PL*Q]
            .reshape(KT // KC, 128, KC * PL * Q)).astype(f8)
        biasr = np.empty((1, PL * Q + 128), dtype=bf)
        biasr[0, :PL * Q] = (b[sl] * SCALE).reshape(-1).astype(bf)
        biasr[0, PL * Q:] = 1.0
        idxq = np.empty((128, (PL + 1) * C), dtype=np.float16)
        idxq[:, 0:C] = iota[:, None]
        idxq[:, C:] = np.broadcast_to(
            part_idx[sl].astype(np.float16).reshape(1, PL * C),
            (128, PL * C))
        in_maps.append({"xT": xT, "biasr": biasr, "ident": ident,
                        "wm": wm, "idxq": idxq})
    return in_maps


def kernel(x, W, b, part_idx, _trace=False):
    from concourse.bass_utils import run_bass_kernel_spmd

    x = np.asarray(x, dtype=np.float32)
    W = np.asarray(W, dtype=np.float32)
    b = np.asarray(b, dtype=np.float32)
    part_idx = np.asarray(part_idx)

    nc = _build_nc()
    in_maps = _host_inputs(x, W, b, part_idx)
    res = run_bass_kernel_spmd(nc, in_maps, list(range(NCORES)),
                               trace=_trace)
    out = np.concatenate(
        [np.asarray(r["out"], dtype=np.float32) for r in res.results], axis=1)
    if _trace:
        return out, res
    return out
